# revision 3
# baseline (speedup 1.0000x reference)
"""Trainium2 Bass kernel for nn_GaussianLayer (segment_reduce).

Computes ll[b, r, k] = -0.5 * sum_d((x[b, regions[r,d]] - means[r,k,d]) / scales[r,k,d])^2
                       - sum_d log(scales[r,k,d]) - 0.5 * D * log(2*pi)

Strategy (data-parallel over batch across 8 cores, 512 rows each):
  Host folds the small [R,K,D] params into matmul weights:
      ll = Xsq @ Wsq + Xraw @ Wraw + const
  where Xraw[b, (r,d)] = x[b, regions[r,d]] (the gather), Xsq = Xraw^2,
  Wsq = -0.5/scales^2, Wraw = means/scales^2 (block-diagonal per region),
  const[r,k] = -0.5*sum_d(means^2/scales^2) - sum_d log(scales) - 0.5*D*log(2pi).

  Device, fully streaming per 128-row batch tile (no HBM scratch):
    DMA x tile [128,1024] f32 -> gpsimd.ap_gather (free-dim gather by
    region order, on-chip) -> ACT cast bf16 -> 8 PE transposes (gathered
    chunk c == matmul lhsT for pair c) -> DVE copy + square ->
    block-diagonal matmuls + const added via k=1 PE matmul -> DVE/ACT
    evacuate PSUM to bf16 -> DMA out.  Output returned bf16, upcast on host.
"""

import os
import sys

for _p in ("/opt/trn_rl_repo", "/root/.axon_site/_ro/trn_rl_repo"):
    if os.path.isdir(_p) and _p not in sys.path:
        sys.path.insert(0, _p)

import numpy as np
import ml_dtypes

import concourse.bass as bass
import concourse.tile as tile
from concourse import bacc, library_config, mybir
from concourse.bass_utils import run_bass_kernel_spmd

LOG_2PI = 1.8378770664093453
B, F = 4096, 1024
R, K, D = 64, 32, 16
NCORES = 8
BL = B // NCORES      # 512 batch rows per core
NT = BL // 128        # 4 batch tiles per core
RKCOLS = R * K        # 2048 output columns
NPAIR = 8             # pair = 8 regions = 128 gathered rows / 256 out cols
N_WARM = 24           # dummy matmuls to ramp the PE p-state early

_module_cache = {}


def _build_module():
    if "nc" in _module_cache:
        return _module_cache["nc"]

    nc = bacc.Bacc(
        trn_type="TRN2",
        target_bir_lowering=False,
        debug=False,
        enable_asserts=False,
    )
    bf16 = mybir.dt.bfloat16
    f32 = mybir.dt.float32
    i16 = mybir.dt.int16

    x_d = nc.dram_tensor("x", [BL, F], f32, kind="ExternalInput").ap()
    wraw_d = nc.dram_tensor("wraw", [128, RKCOLS], bf16, kind="ExternalInput").ap()
    wsq_d = nc.dram_tensor("wsq", [128, RKCOLS], bf16, kind="ExternalInput").ap()
    cstb_d = nc.dram_tensor("cstb", [1, RKCOLS], bf16, kind="ExternalInput").ap()
    ones_d = nc.dram_tensor("ones", [1, 128], bf16, kind="ExternalInput").ap()
    idx_d = nc.dram_tensor("idx", [128, F // 16], i16, kind="ExternalInput").ap()
    id_d = nc.dram_tensor("ident", [128, 128], bf16, kind="ExternalInput").ap()
    out_d = nc.dram_tensor("out", [BL, RKCOLS], bf16, kind="ExternalOutput").ap()

    with tile.TileContext(nc) as tc:
        with (
            tc.tile_pool(name="persist", bufs=1) as persist,
            tc.tile_pool(name="xin", bufs=4) as xpool,
            tc.tile_pool(name="xg", bufs=2) as xgpool,
            tc.tile_pool(name="xgb", bufs=2) as xgbpool,
            tc.tile_pool(name="trp", bufs=2, space="PSUM") as trpool,
            tc.tile_pool(name="wrm", bufs=1, space="PSUM") as warmpool,
            tc.tile_pool(name="gt", bufs=2) as gtpool,
            tc.tile_pool(name="sq", bufs=2) as sqpool,
            tc.tile_pool(name="po", bufs=4, space="PSUM") as popool,
            tc.tile_pool(name="osb", bufs=2) as opool,
        ):
            nc.gpsimd.load_library(library_config.ap_gather)

            # --- input DMAs, priority order: x tile 0 gates everything ---
            xts = []
            xt0 = xpool.tile([128, F], f32, tag="x0")
            nc.sync.dma_start(xt0[:], x_d[0:128, :])
            xts.append(xt0)

            idx = persist.tile([128, F // 16], i16)
            nc.sync.dma_start(idx[:], idx_d)
            ident = persist.tile([128, 128], bf16)
            nc.sync.dma_start(ident[:], id_d)
            cstb = persist.tile([1, RKCOLS], bf16)
            nc.sync.dma_start(cstb[:], cstb_d)
            ones = persist.tile([1, 128], bf16)
            nc.sync.dma_start(ones[:], ones_d)

            xt1 = xpool.tile([128, F], f32, tag="x1")
            nc.sync.dma_start(xt1[:], x_d[128:256, :])
            xts.append(xt1)

            w_raw = persist.tile([128, RKCOLS], bf16)
            nc.sync.dma_start(w_raw[:], wraw_d)
            w_sq = persist.tile([128, RKCOLS], bf16)
            nc.sync.dma_start(w_sq[:], wsq_d)

            for bt in (2, 3):
                xt = xpool.tile([128, F], f32, tag=f"x{bt}")
                nc.sync.dma_start(xt[:], x_d[bt * 128:(bt + 1) * 128, :])
                xts.append(xt)

            # --- PE warm-up: ramp the p-state while the pipeline fills ---
            warm = warmpool.tile([128, 512], f32)
            for i in range(N_WARM):
                nc.tensor.matmul(
                    warm[:, (i % 4) * 128:(i % 4) * 128 + 128],
                    ident[:], ident[:], start=True, stop=True,
                )

            # --- streaming pipeline over the 4 batch tiles ---
            for bt in range(NT):
                rs = slice(bt * 128, (bt + 1) * 128)

                xg = xgpool.tile([128, F], f32)
                nc.gpsimd.ap_gather(
                    out_ap=xg[:],
                    in_ap=xts[bt][:],
                    idxs_ap=idx[:],
                    channels=128,
                    num_elems=F,
                    d=1,
                    num_idxs=F,
                )
                xgb = xgbpool.tile([128, F], bf16)
                nc.scalar.copy(xgb[:], xg[:])

                gt = gtpool.tile([128, F], bf16)
                sq = sqpool.tile([128, F], bf16)
                for half in range(2):
                    pt = trpool.tile([128, 512], bf16)
                    for jj in range(4):
                        c = 4 * half + jj
                        nc.tensor.transpose(
                            pt[:, jj * 128:(jj + 1) * 128],
                            xgb[:, c * 128:(c + 1) * 128],
                            ident[:],
                        )
                    hs = slice(half * 512, (half + 1) * 512)
                    nc.vector.tensor_copy(gt[:, hs], pt[:])
                    nc.vector.tensor_mul(sq[:, hs], gt[:, hs], gt[:, hs])

                osb = opool.tile([128, RKCOLS], bf16)
                for q in range(4):
                    po = popool.tile([128, 512], f32)
                    for h in range(2):
                        p = 2 * q + h
                        co = slice(h * 256, (h + 1) * 256)
                        wc = slice(p * 256, (p + 1) * 256)
                        ps = slice(p * 128, (p + 1) * 128)
                        nc.tensor.matmul(
                            po[:, co], gt[:, ps], w_raw[:, wc],
                            start=True, stop=False,
                        )
                        nc.tensor.matmul(
                            po[:, co], sq[:, ps], w_sq[:, wc],
                            start=False, stop=False,
                        )
                        nc.tensor.matmul(
                            po[:, co], ones[:],
                            cstb[:, q * 512 + h * 256:q * 512 + (h + 1) * 256],
                            start=False, stop=True,
                        )
                    cs = slice(q * 512, (q + 1) * 512)
                    if q % 2 == 0:
                        nc.vector.tensor_copy(osb[:, cs], po[:])
                    else:
                        nc.scalar.copy(osb[:, cs], po[:])
                    if q == 1:
                        nc.gpsimd.dma_start(out_d[rs, 0:1024], osb[:, 0:1024])
                nc.gpsimd.dma_start(out_d[rs, 1024:2048], osb[:, 1024:2048])

    nc.compile()
    _module_cache["nc"] = nc
    return nc


def _prep_params(regions, means, scales):
    """Host folding of the small [R,K,D] params into matmul weights."""
    regions = np.asarray(regions).astype(np.int64)
    means = np.asarray(means, dtype=np.float64)
    scales = np.asarray(scales, dtype=np.float64)

    inv2 = 1.0 / scales**2                                   # [R,K,D]
    wsq_c = -0.5 * inv2                                      # coeff of x^2
    wraw_c = means * inv2                                    # coeff of x
    const = (
        -0.5 * np.sum(means**2 * inv2, axis=-1)
        - np.sum(np.log(scales), axis=-1)
        - 0.5 * D * LOG_2PI
    )                                                        # [R,K]

    # Block-diagonal weight tiles: pair p covers regions 8p..8p+7.
    # Row 16j+d (region-local j in 0..7), col 32j+k.
    wraw = np.zeros((128, RKCOLS), np.float32)
    wsq = np.zeros((128, RKCOLS), np.float32)
    for p in range(NPAIR):
        for j in range(8):
            r = 8 * p + j
            rows = slice(16 * j, 16 * j + 16)
            cols = slice(256 * p + 32 * j, 256 * p + 32 * j + 32)
            wraw[rows, cols] = wraw_c[r].T.astype(np.float32)   # [D, K]
            wsq[rows, cols] = wsq_c[r].T.astype(np.float32)
    wraw = wraw.astype(ml_dtypes.bfloat16)
    wsq = wsq.astype(ml_dtypes.bfloat16)

    cstb = const.reshape(1, -1).astype(ml_dtypes.bfloat16).copy()
    ones = np.ones((1, 128), ml_dtypes.bfloat16)

    # ap_gather index layout: index j of the 1024-wide gather lives at
    # [j % 16, j // 16], replicated across the eight 16-partition groups.
    perm = regions.reshape(-1).astype(np.int16)              # [1024]
    idx16 = perm.reshape(F // 16, 16).T                      # [16, 64]
    idx = np.tile(idx16, (8, 1)).copy()                      # [128, 64]

    ident = np.eye(128, dtype=ml_dtypes.bfloat16)
    return wraw, wsq, cstb, ones, idx, ident


def _run(inputs, trace=False, **kwargs):
    x = np.ascontiguousarray(np.asarray(inputs["x"], dtype=np.float32))
    assert x.shape == (B, F), x.shape
    wraw, wsq, cstb, ones, idx, ident = _prep_params(
        inputs["regions"], inputs["means"], inputs["scales"]
    )

    nc = _build_module()
    in_maps = []
    for c in range(NCORES):
        in_maps.append({
            "x": np.ascontiguousarray(x[c * BL:(c + 1) * BL]),
            "wraw": wraw,
            "wsq": wsq,
            "cstb": cstb,
            "ones": ones,
            "idx": idx,
            "ident": ident,
        })
    res = run_bass_kernel_spmd(
        nc, in_maps, core_ids=list(range(NCORES)), trace=trace, **kwargs
    )
    out = np.concatenate(
        [np.asarray(res.results[c]["out"]).astype(np.float32)
         for c in range(NCORES)], axis=0
    ).reshape(B, R, K)
    return out, res


def kernel(**inputs):
    out, _ = _run(inputs, trace=False)
    return out


# revision 5
# speedup vs baseline: 1.6762x; 1.6762x over previous
"""Trainium2 Bass kernel for nn_GaussianLayer (segment_reduce).

Computes ll[b, r, k] = -0.5 * sum_d((x[b, regions[r,d]] - means[r,k,d]) / scales[r,k,d])^2
                       - sum_d log(scales[r,k,d]) - 0.5 * D * log(2*pi)

Strategy (data-parallel over batch across 8 cores, 512 rows each):
  Host folds the small [R,K,D] params into matmul weights:
      ll = Xsq @ Wsq + Xraw @ Wraw + const
  where Xraw[b, (r,d)] = x[b, regions[r,d]] (the gather), Xsq = Xraw^2,
  Wsq = -0.5/scales^2, Wraw = means/scales^2 (block-diagonal per region),
  const[r,k] = -0.5*sum_d(means^2/scales^2) - sum_d log(scales) - 0.5*D*log(2pi).

  Device, streaming per 128-row batch tile.  The feature gather is done
  with hardware-DGE indirect DMAs (offset table in SBUF) -- no gpsimd
  Q7 library, whose load latency (~30us) would dominate the kernel:
    DMA x tile -> ACT cast bf16 -> 8 PE transposes -> DVE copy to SBUF ->
    DMA to per-tile HBM scratch xT [1024 feat, 128 batch] ->
    8x indirect_dma_start row-gathers (region order) -> lhsT tiles ->
    DVE square -> block-diag matmuls + const via k=1 PE matmul ->
    DVE/ACT evacuate PSUM to bf16 -> DMA out.  Output bf16, host upcast.
"""

import os
import sys

for _p in ("/opt/trn_rl_repo", "/root/.axon_site/_ro/trn_rl_repo"):
    if os.path.isdir(_p) and _p not in sys.path:
        sys.path.insert(0, _p)

import numpy as np
import ml_dtypes

import concourse.bass as bass
import concourse.tile as tile
from concourse import bacc, library_config, mybir
from concourse.bass_utils import run_bass_kernel_spmd

LOG_2PI = 1.8378770664093453
B, F = 4096, 1024
R, K, D = 64, 32, 16
NCORES = 8
BL = B // NCORES      # 512 batch rows per core
NT = BL // 128        # 4 batch tiles per core
RKCOLS = R * K        # 2048 output columns
NPAIR = 8             # pair = 8 regions = 128 gathered rows / 256 out cols
N_WARM = 24           # dummy matmuls to ramp the PE p-state early

_module_cache = {}


def _build_module():
    if "nc" in _module_cache:
        return _module_cache["nc"]

    nc = bacc.Bacc(
        trn_type="TRN2",
        target_bir_lowering=False,
        debug=False,
        enable_asserts=False,
    )
    bf16 = mybir.dt.bfloat16
    f32 = mybir.dt.float32
    i32 = mybir.dt.int32

    x_d = nc.dram_tensor("x", [BL, F], f32, kind="ExternalInput").ap()
    wraw_d = nc.dram_tensor("wraw", [128, RKCOLS], bf16, kind="ExternalInput").ap()
    wsq_d = nc.dram_tensor("wsq", [128, RKCOLS], bf16, kind="ExternalInput").ap()
    cstb_d = nc.dram_tensor("cstb", [1, RKCOLS], bf16, kind="ExternalInput").ap()
    offs_d = nc.dram_tensor("offs", [128, NPAIR], i32, kind="ExternalInput").ap()
    id_d = nc.dram_tensor("ident", [128, 128], bf16, kind="ExternalInput").ap()
    out_d = nc.dram_tensor("out", [BL, RKCOLS], bf16, kind="ExternalOutput").ap()

    with tile.TileContext(nc) as tc:
        with (
            tc.tile_pool(name="persist", bufs=1) as persist,
            tc.tile_pool(name="dram", bufs=1, space="DRAM") as drampool,
            tc.tile_pool(name="xin", bufs=3) as xpool,
            tc.tile_pool(name="xgb", bufs=2) as xgbpool,
            tc.tile_pool(name="trp", bufs=2, space="PSUM") as trpool,
            tc.tile_pool(name="wrm", bufs=1, space="PSUM") as warmpool,
            tc.tile_pool(name="xts", bufs=2) as xtspool,
            tc.tile_pool(name="gt", bufs=2) as gtpool,
            tc.tile_pool(name="sq", bufs=2) as sqpool,
            tc.tile_pool(name="po", bufs=4, space="PSUM") as popool,
            tc.tile_pool(name="osb", bufs=2) as opool,
        ):
            # --- input DMAs, priority order: x tile 0 gates everything ---
            xts_in = []
            xt0 = xpool.tile([128, F], f32, tag="x0")
            nc.sync.dma_start(xt0[:], x_d[0:128, :])
            xts_in.append(xt0)

            offs = persist.tile([128, NPAIR], i32)
            nc.sync.dma_start(offs[:], offs_d)
            ident = persist.tile([128, 128], bf16)
            nc.sync.dma_start(ident[:], id_d)
            cstb = persist.tile([1, RKCOLS], bf16)
            nc.sync.dma_start(cstb[:], cstb_d)

            xt1 = xpool.tile([128, F], f32, tag="x1")
            nc.sync.dma_start(xt1[:], x_d[128:256, :])
            xts_in.append(xt1)

            w_raw = persist.tile([128, RKCOLS], bf16)
            nc.sync.dma_start(w_raw[:], wraw_d)
            w_sq = persist.tile([128, RKCOLS], bf16)
            nc.sync.dma_start(w_sq[:], wsq_d)

            for bt in (2, 3):
                xt = xpool.tile([128, F], f32, tag=f"x{bt}")
                nc.sync.dma_start(xt[:], x_d[bt * 128:(bt + 1) * 128, :])
                xts_in.append(xt)

            ones = persist.tile([1, 128], bf16)
            nc.vector.memset(ones[:], 1.0)

            # per-tile HBM scratch: xT for tile bt, row f = 128 batch vals
            xtd = []
            for bt in range(NT):
                xtd_t = drampool.tile([F, 128], bf16, tag=f"xtd{bt}")
                xtd.append(xtd_t)

            # --- PE warm-up: ramp the p-state while the pipeline fills ---
            warm = warmpool.tile([128, 512], f32)
            for i in range(N_WARM):
                nc.tensor.matmul(
                    warm[:, (i % 4) * 128:(i % 4) * 128 + 128],
                    ident[:], ident[:], start=True, stop=True,
                )

            # --- streaming pipeline over the 4 batch tiles ---
            for bt in range(NT):
                rs = slice(bt * 128, (bt + 1) * 128)

                xgb = xgbpool.tile([128, F], bf16)
                nc.scalar.copy(xgb[:], xts_in[bt][:])

                xts = xtspool.tile([128, F], bf16)  # [128, 8 chunks, 128 b]
                for half in range(2):
                    pt = trpool.tile([128, 512], bf16)
                    for jj in range(4):
                        c = 4 * half + jj
                        nc.tensor.transpose(
                            pt[:, jj * 128:(jj + 1) * 128],
                            xgb[:, c * 128:(c + 1) * 128],
                            ident[:],
                        )
                    nc.vector.tensor_copy(
                        xts[:, half * 512:(half + 1) * 512], pt[:]
                    )
                # write natural-feature-order xT for this tile to HBM
                nc.gpsimd.dma_start(
                    xtd[bt][:].rearrange("(c p) b -> p c b", p=128),
                    xts[:].rearrange("p (c b) -> p c b", c=8),
                )

                # hardware-DGE row gathers: lhsT chunk p = pair p's features
                gt = gtpool.tile([128, F], bf16)
                for p in range(NPAIR):
                    nc.gpsimd.indirect_dma_start(
                        out=gt[:, p * 128:(p + 1) * 128],
                        out_offset=None,
                        in_=xtd[bt][:],
                        in_offset=bass.IndirectOffsetOnAxis(
                            ap=offs[:, p:p + 1], axis=0,
                        ),
                    )

                sq = sqpool.tile([128, F], bf16)
                for half in range(2):
                    hs = slice(half * 512, (half + 1) * 512)
                    nc.vector.tensor_mul(sq[:, hs], gt[:, hs], gt[:, hs])

                osb = opool.tile([128, RKCOLS], bf16)
                for q in range(4):
                    po = popool.tile([128, 512], f32)
                    for h in range(2):
                        p = 2 * q + h
                        co = slice(h * 256, (h + 1) * 256)
                        wc = slice(p * 256, (p + 1) * 256)
                        ps = slice(p * 128, (p + 1) * 128)
                        nc.tensor.matmul(
                            po[:, co], gt[:, ps], w_raw[:, wc],
                            start=True, stop=False,
                        )
                        nc.tensor.matmul(
                            po[:, co], sq[:, ps], w_sq[:, wc],
                            start=False, stop=False,
                        )
                        nc.tensor.matmul(
                            po[:, co], ones[:],
                            cstb[:, q * 512 + h * 256:q * 512 + (h + 1) * 256],
                            start=False, stop=True,
                        )
                    cs = slice(q * 512, (q + 1) * 512)
                    if q % 2 == 0:
                        nc.vector.tensor_copy(osb[:, cs], po[:])
                    else:
                        nc.scalar.copy(osb[:, cs], po[:])
                    if q == 1:
                        nc.sync.dma_start(out_d[rs, 0:1024], osb[:, 0:1024])
                nc.sync.dma_start(out_d[rs, 1024:2048], osb[:, 1024:2048])

    nc.compile()
    _module_cache["nc"] = nc
    return nc


def _prep_params(regions, means, scales):
    """Host folding of the small [R,K,D] params into matmul weights."""
    regions = np.asarray(regions).astype(np.int64)
    means = np.asarray(means, dtype=np.float64)
    scales = np.asarray(scales, dtype=np.float64)

    inv2 = 1.0 / scales**2                                   # [R,K,D]
    wsq_c = -0.5 * inv2                                      # coeff of x^2
    wraw_c = means * inv2                                    # coeff of x
    const = (
        -0.5 * np.sum(means**2 * inv2, axis=-1)
        - np.sum(np.log(scales), axis=-1)
        - 0.5 * D * LOG_2PI
    )                                                        # [R,K]

    # Block-diagonal weight tiles: pair p covers regions 8p..8p+7.
    # Row 16j+d (region-local j in 0..7), col 32j+k.
    wraw = np.zeros((128, RKCOLS), np.float32)
    wsq = np.zeros((128, RKCOLS), np.float32)
    for p in range(NPAIR):
        for j in range(8):
            r = 8 * p + j
            rows = slice(16 * j, 16 * j + 16)
            cols = slice(256 * p + 32 * j, 256 * p + 32 * j + 32)
            wraw[rows, cols] = wraw_c[r].T.astype(np.float32)   # [D, K]
            wsq[rows, cols] = wsq_c[r].T.astype(np.float32)
    wraw = wraw.astype(ml_dtypes.bfloat16)
    wsq = wsq.astype(ml_dtypes.bfloat16)

    cstb = const.reshape(1, -1).astype(ml_dtypes.bfloat16).copy()

    # indirect-DMA offset table: gather j of lhsT chunk p reads xT row
    # perm[128p + j]; offsets live one-per-partition in column p.
    perm = regions.reshape(-1).astype(np.int32)              # [1024]
    offs = perm.reshape(NPAIR, 128).T.copy()                 # [128, 8]

    ident = np.eye(128, dtype=ml_dtypes.bfloat16)
    return wraw, wsq, cstb, offs, ident


def _run(inputs, trace=False, **kwargs):
    x = np.ascontiguousarray(np.asarray(inputs["x"], dtype=np.float32))
    assert x.shape == (B, F), x.shape
    wraw, wsq, cstb, offs, ident = _prep_params(
        inputs["regions"], inputs["means"], inputs["scales"]
    )

    nc = _build_module()
    in_maps = []
    for c in range(NCORES):
        in_maps.append({
            "x": np.ascontiguousarray(x[c * BL:(c + 1) * BL]),
            "wraw": wraw,
            "wsq": wsq,
            "cstb": cstb,
            "offs": offs,
            "ident": ident,
        })
    res = run_bass_kernel_spmd(
        nc, in_maps, core_ids=list(range(NCORES)), trace=trace, **kwargs
    )
    out = np.concatenate(
        [np.asarray(res.results[c]["out"]).astype(np.float32)
         for c in range(NCORES)], axis=0
    ).reshape(B, R, K)
    return out, res


def kernel(**inputs):
    out, _ = _run(inputs, trace=False)
    return out


# revision 8
# speedup vs baseline: 2.0402x; 1.2172x over previous
"""Trainium2 Bass kernel for nn_GaussianLayer (segment_reduce).

Computes ll[b, r, k] = -0.5 * sum_d((x[b, regions[r,d]] - means[r,k,d]) / scales[r,k,d])^2
                       - sum_d log(scales[r,k,d]) - 0.5 * D * log(2*pi)

Strategy (data-parallel over batch across 8 cores, 512 rows each):
  Host folds the small [R,K,D] params into matmul weights:
      ll = Xsq @ Wsq + Xraw @ Wraw + const
  where Xraw[b, (r,d)] = x[b, regions[r,d]] (the gather), Xsq = Xraw^2,
  Wsq = -0.5/scales^2, Wraw = means/scales^2 (block-diagonal per region),
  const[r,k] = -0.5*sum_d(means^2/scales^2) - sum_d log(scales) - 0.5*D*LOG2PI.

  Device: phase 1 streams the 4 batch tiles (DMA x -> ACT cast bf16 ->
  8 PE transposes -> DVE copy -> HWDGE DMA into a full HBM xT scratch
  [1024 feat, 512 batch] bf16).  The feature gather is 8 indirect DMAs
  (one per region-pair, offset table [128,1] per gather = one xT row per
  partition) -- hardware DGE + SWDGE descriptor generation, but NO
  gpsimd Q7 library, whose ~30us load latency would dominate.  Phase 3
  runs bank-major so matmuls chase the gathers pair by pair; const is
  accumulated via k=1 PE matmuls; PSUM is evacuated to bf16 by DVE/ACT
  and DMA'd out.  Output bf16, host upcast.
"""

import os
import sys

for _p in ("/opt/trn_rl_repo", "/root/.axon_site/_ro/trn_rl_repo"):
    if os.path.isdir(_p) and _p not in sys.path:
        sys.path.insert(0, _p)

import numpy as np
import ml_dtypes

import concourse.bass as bass
import concourse.tile as tile
from concourse import bacc, library_config, mybir
from concourse.bass_utils import run_bass_kernel_spmd

LOG_2PI = 1.8378770664093453
B, F = 4096, 1024
R, K, D = 64, 32, 16
NCORES = 8
BL = B // NCORES      # 512 batch rows per core
NT = BL // 128        # 4 batch tiles per core
RKCOLS = R * K        # 2048 output columns
NPAIR = 8             # pair = 8 regions = 128 gathered rows / 256 out cols
N_WARM = 24           # dummy matmuls to ramp the PE p-state early

_module_cache = {}


def _build_module():
    if "nc" in _module_cache:
        return _module_cache["nc"]

    nc = bacc.Bacc(
        trn_type="TRN2",
        target_bir_lowering=False,
        debug=False,
        enable_asserts=False,
    )
    bf16 = mybir.dt.bfloat16
    f32 = mybir.dt.float32
    i32 = mybir.dt.int32

    x_d = nc.dram_tensor("x", [BL, F], f32, kind="ExternalInput").ap()
    wraw_d = nc.dram_tensor("wraw", [128, RKCOLS], bf16, kind="ExternalInput").ap()
    wsq_d = nc.dram_tensor("wsq", [128, RKCOLS], bf16, kind="ExternalInput").ap()
    cstb_d = nc.dram_tensor("cstb", [1, RKCOLS], bf16, kind="ExternalInput").ap()
    offs_d = nc.dram_tensor("offs", [128, NPAIR], i32, kind="ExternalInput").ap()
    id_d = nc.dram_tensor("ident", [128, 128], bf16, kind="ExternalInput").ap()
    out_d = nc.dram_tensor("out", [BL, RKCOLS], bf16, kind="ExternalOutput").ap()

    with tile.TileContext(nc) as tc:
        with (
            tc.tile_pool(name="persist", bufs=1) as persist,
            tc.tile_pool(name="dram", bufs=1, space="DRAM") as drampool,
            tc.tile_pool(name="xin", bufs=4) as xpool,
            tc.tile_pool(name="xgb", bufs=2) as xgbpool,
            tc.tile_pool(name="trp", bufs=2, space="PSUM") as trpool,
            tc.tile_pool(name="wrm", bufs=1, space="PSUM") as warmpool,
            tc.tile_pool(name="xts", bufs=2) as xtspool,
            tc.tile_pool(name="gt", bufs=1) as gtpool,
            tc.tile_pool(name="sq", bufs=1) as sqpool,
            tc.tile_pool(name="po", bufs=4, space="PSUM") as popool,
            tc.tile_pool(name="osb", bufs=1) as opool,
        ):
            # --- input DMAs: x tiles first (they gate the xT barrier),
            # small persists next, the 1MB weights last (needed ~15us) ---
            xts_in = []
            xt0 = xpool.tile([128, F], f32, tag="x0")
            nc.sync.dma_start(xt0[:], x_d[0:128, :])
            xts_in.append(xt0)

            ident = persist.tile([128, 128], bf16)
            nc.sync.dma_start(ident[:], id_d)
            offs = persist.tile([128, NPAIR], i32)
            nc.sync.dma_start(offs[:], offs_d)
            cstb = persist.tile([1, RKCOLS], bf16)
            nc.sync.dma_start(cstb[:], cstb_d)

            for bt in (1, 2, 3):
                xt = xpool.tile([128, F], f32, tag=f"x{bt}")
                nc.sync.dma_start(xt[:], x_d[bt * 128:(bt + 1) * 128, :])
                xts_in.append(xt)

            w_raw = persist.tile([128, RKCOLS], bf16)
            nc.sync.dma_start(w_raw[:], wraw_d)
            w_sq = persist.tile([128, RKCOLS], bf16)
            nc.sync.dma_start(w_sq[:], wsq_d)

            ones = persist.tile([1, 128], bf16)
            nc.vector.memset(ones[:], 1.0)

            # full HBM scratch: xT row f = feature f, 512 batch cols bf16
            xt_dram = drampool.tile([F, BL], bf16)
            xt_wview = xt_dram[:].rearrange("(c p) b -> p c b", p=128)

            # --- PE warm-up: ramp the p-state while the pipeline fills ---
            warm = warmpool.tile([128, 512], f32)
            for i in range(N_WARM):
                nc.tensor.matmul(
                    warm[:, (i % 4) * 128:(i % 4) * 128 + 128],
                    ident[:], ident[:], start=True, stop=True,
                )

            # --- phase 1: stream the 4 batch tiles into the xT scratch ---
            for bt in range(NT):
                xgb = xgbpool.tile([128, F], bf16)
                nc.scalar.copy(xgb[:], xts_in[bt][:])

                xts = xtspool.tile([128, F], bf16)
                for half in range(2):
                    pt = trpool.tile([128, 512], bf16)
                    for jj in range(4):
                        c = 4 * half + jj
                        nc.tensor.transpose(
                            pt[:, jj * 128:(jj + 1) * 128],
                            xgb[:, c * 128:(c + 1) * 128],
                            ident[:],
                        )
                    nc.vector.tensor_copy(
                        xts[:, half * 512:(half + 1) * 512], pt[:]
                    )
                nc.sync.dma_start(
                    xt_wview[:, :, bt * 128:(bt + 1) * 128],
                    xts[:].rearrange("p (c b) -> p c b", c=8),
                )

            # --- phase 2: 8 per-pair indirect row gathers (full batch) ---
            gts = []
            for p in range(NPAIR):
                gt = gtpool.tile([128, BL], bf16, tag=f"gt{p}")
                nc.gpsimd.indirect_dma_start(
                    out=gt[:],
                    out_offset=None,
                    in_=xt_dram[:],
                    in_offset=bass.IndirectOffsetOnAxis(
                        ap=offs[:, p:p + 1], axis=0,
                    ),
                )
                gts.append(gt)

            # --- phase 3: bank-major matmuls chase the gathers ---
            osbs = []
            for bt in range(NT):
                osb = opool.tile([128, RKCOLS], bf16, tag=f"osb{bt}")
                osbs.append(osb)
            sqs = {}
            for q in range(4):
                for h in range(2):
                    p = 2 * q + h
                    sq = sqpool.tile([128, BL], bf16, tag=f"sq{p}")
                    nc.vector.tensor_mul(sq[:], gts[p][:], gts[p][:])
                    sqs[p] = sq
                for bt in range(NT):
                    rs = slice(bt * 128, (bt + 1) * 128)
                    bs = slice(bt * 128, (bt + 1) * 128)
                    po = popool.tile([128, 512], f32)
                    for h in range(2):
                        p = 2 * q + h
                        co = slice(h * 256, (h + 1) * 256)
                        wc = slice(p * 256, (p + 1) * 256)
                        nc.tensor.matmul(
                            po[:, co], gts[p][:, bs], w_raw[:, wc],
                            start=True, stop=False,
                        )
                        nc.tensor.matmul(
                            po[:, co], sqs[p][:, bs], w_sq[:, wc],
                            start=False, stop=False,
                        )
                        nc.tensor.matmul(
                            po[:, co], ones[:],
                            cstb[:, q * 512 + h * 256:q * 512 + (h + 1) * 256],
                            start=False, stop=True,
                        )
                    cs = slice(q * 512, (q + 1) * 512)
                    if (q + bt) % 2 == 0:
                        nc.vector.tensor_copy(osbs[bt][:, cs], po[:])
                    else:
                        nc.scalar.copy(osbs[bt][:, cs], po[:])
                    if q == 1:
                        nc.sync.dma_start(
                            out_d[rs, 0:1024], osbs[bt][:, 0:1024]
                        )
                    if q == 3:
                        nc.sync.dma_start(
                            out_d[rs, 1024:2048], osbs[bt][:, 1024:2048]
                        )

    nc.compile()
    _module_cache["nc"] = nc
    return nc


def _prep_params(regions, means, scales):
    """Host folding of the small [R,K,D] params into matmul weights."""
    regions = np.asarray(regions).astype(np.int64)
    means = np.asarray(means, dtype=np.float64)
    scales = np.asarray(scales, dtype=np.float64)

    inv2 = 1.0 / scales**2                                   # [R,K,D]
    wsq_c = -0.5 * inv2                                      # coeff of x^2
    wraw_c = means * inv2                                    # coeff of x
    const = (
        -0.5 * np.sum(means**2 * inv2, axis=-1)
        - np.sum(np.log(scales), axis=-1)
        - 0.5 * D * LOG_2PI
    )                                                        # [R,K]

    # Block-diagonal weight tiles: pair p covers regions 8p..8p+7.
    # Row 16j+d (region-local j in 0..7), col 32j+k.
    wraw = np.zeros((128, RKCOLS), np.float32)
    wsq = np.zeros((128, RKCOLS), np.float32)
    for p in range(NPAIR):
        for j in range(8):
            r = 8 * p + j
            rows = slice(16 * j, 16 * j + 16)
            cols = slice(256 * p + 32 * j, 256 * p + 32 * j + 32)
            wraw[rows, cols] = wraw_c[r].T.astype(np.float32)   # [D, K]
            wsq[rows, cols] = wsq_c[r].T.astype(np.float32)
    wraw = wraw.astype(ml_dtypes.bfloat16)
    wsq = wsq.astype(ml_dtypes.bfloat16)

    cstb = const.reshape(1, -1).astype(ml_dtypes.bfloat16).copy()

    # per-pair gather offsets: gather p, partition j reads xT row
    # perm[128p + j]; table column p holds those offsets.
    perm = regions.reshape(-1).astype(np.int64)              # [1024]
    offs = perm.reshape(NPAIR, 128).T.astype(np.int32).copy()  # [128, 8]

    ident = np.eye(128, dtype=ml_dtypes.bfloat16)
    return wraw, wsq, cstb, offs, ident


def _run(inputs, trace=False, **kwargs):
    x = np.ascontiguousarray(np.asarray(inputs["x"], dtype=np.float32))
    assert x.shape == (B, F), x.shape
    wraw, wsq, cstb, offs, ident = _prep_params(
        inputs["regions"], inputs["means"], inputs["scales"]
    )

    nc = _build_module()
    in_maps = []
    for c in range(NCORES):
        in_maps.append({
            "x": np.ascontiguousarray(x[c * BL:(c + 1) * BL]),
            "wraw": wraw,
            "wsq": wsq,
            "cstb": cstb,
            "offs": offs,
            "ident": ident,
        })
    res = run_bass_kernel_spmd(
        nc, in_maps, core_ids=list(range(NCORES)), trace=trace, **kwargs
    )
    out = np.concatenate(
        [np.asarray(res.results[c]["out"]).astype(np.float32)
         for c in range(NCORES)], axis=0
    ).reshape(B, R, K)
    return out, res


def kernel(**inputs):
    out, _ = _run(inputs, trace=False)
    return out


# revision 13
# speedup vs baseline: 2.5884x; 1.2687x over previous
"""Trainium2 Bass kernel for nn_GaussianLayer (segment_reduce).

Computes ll[b, r, k] = -0.5 * sum_d((x[b, regions[r,d]] - means[r,k,d]) / scales[r,k,d])^2
                       - sum_d log(scales[r,k,d]) - 0.5 * D * log(2*pi)

Strategy (data-parallel over batch across 8 cores, 512 rows each):
  Host folds the small [R,K,D] params into matmul weights:
      ll = Xsq @ Wsq + Xraw @ Wraw + const
  where Xraw[b, (r,d)] = x[b, regions[r,d]] (the gather), Xsq = Xraw^2,
  Wsq = -0.5/scales^2, Wraw = means/scales^2 (block-diagonal per region),
  const[r,k] = -0.5*sum_d(means^2/scales^2) - sum_d log(scales) - 0.5*D*LOG2PI.

  Device: phase 1 streams the 4 batch tiles (DMA x -> ACT cast bf16 ->
  8 PE transposes -> HWDGE DMA straight from PSUM into a full HBM xT
  scratch [1024 feat, 512 batch] bf16).  The feature gather is 8
  indirect DMAs (one per region-pair, offset table [128,1] = one xT row
  per partition) -- no gpsimd Q7 library, whose ~30us load latency
  would dominate.  const is broadcast across partitions once via k=1 PE
  matmuls into PSUM and copied to SBUF; phase 3 runs 2-matmul
  accumulation groups (raw+sq) into 2-bank PSUM tiles which DVE
  evacuates with a fused (po + const -> bf16) scalar_tensor_tensor.
  Output bf16, host upcast.
"""

import os
import sys

for _p in ("/opt/trn_rl_repo", "/root/.axon_site/_ro/trn_rl_repo"):
    if os.path.isdir(_p) and _p not in sys.path:
        sys.path.insert(0, _p)

import numpy as np
import ml_dtypes

import concourse.bass as bass
import concourse.tile as tile
from concourse import bacc, library_config, mybir
from concourse.bass_utils import run_bass_kernel_spmd

LOG_2PI = 1.8378770664093453
B, F = 4096, 1024
R, K, D = 64, 32, 16
NCORES = 8
BL = B // NCORES      # 512 batch rows per core
NT = BL // 128        # 4 batch tiles per core
RKCOLS = R * K        # 2048 output columns
NPAIR = 8             # pair = 8 regions = 128 gathered rows / 256 out cols

_module_cache = {}


def _build_module():
    if "nc" in _module_cache:
        return _module_cache["nc"]

    nc = bacc.Bacc(
        trn_type="TRN2",
        target_bir_lowering=False,
        debug=False,
        enable_asserts=False,
    )
    bf16 = mybir.dt.bfloat16
    f32 = mybir.dt.float32
    i32 = mybir.dt.int32

    x_d = nc.dram_tensor("x", [BL, F], f32, kind="ExternalInput").ap()
    wraw_d = nc.dram_tensor("wraw", [128, RKCOLS], bf16, kind="ExternalInput").ap()
    wsq_d = nc.dram_tensor("wsq", [128, RKCOLS], bf16, kind="ExternalInput").ap()
    cstb_d = nc.dram_tensor("cstb", [1, RKCOLS], bf16, kind="ExternalInput").ap()
    offs_d = nc.dram_tensor("offs", [128, NPAIR], i32, kind="ExternalInput").ap()
    id_d = nc.dram_tensor("ident", [128, 128], bf16, kind="ExternalInput").ap()
    out_d = nc.dram_tensor("out", [BL, RKCOLS], bf16, kind="ExternalOutput").ap()

    with tile.TileContext(nc) as tc:
        with (
            tc.tile_pool(name="persist", bufs=1) as persist,
            tc.tile_pool(name="dram", bufs=1, space="DRAM") as drampool,
            tc.tile_pool(name="xin", bufs=4) as xpool,
            tc.tile_pool(name="xgb", bufs=2) as xgbpool,
            tc.tile_pool(name="trp", bufs=2, space="PSUM") as trpool,
            tc.tile_pool(name="xts", bufs=2) as xtspool,
            tc.tile_pool(name="gt", bufs=1) as gtpool,
            tc.tile_pool(name="sq", bufs=1) as sqpool,
            tc.tile_pool(name="cstp", bufs=1, space="PSUM") as cstpool,
            tc.tile_pool(name="po", bufs=4, space="PSUM") as popool,
            tc.tile_pool(name="osb", bufs=1) as opool,
        ):
            # --- input DMAs: x tiles first (they gate the xT barrier),
            # small persists next, the 1MB weights last (needed ~15us) ---
            xts_in = []
            xt0 = xpool.tile([128, F], f32, tag="x0")
            nc.sync.dma_start(xt0[:], x_d[0:128, :])
            xts_in.append(xt0)

            ident = persist.tile([128, 128], bf16)
            nc.sync.dma_start(ident[:], id_d)
            offs = persist.tile([128, NPAIR], i32)
            nc.sync.dma_start(offs[:], offs_d)
            cstb = persist.tile([1, RKCOLS], bf16)
            nc.sync.dma_start(cstb[:], cstb_d)

            for bt in (1, 2, 3):
                xt = xpool.tile([128, F], f32, tag=f"x{bt}")
                nc.sync.dma_start(xt[:], x_d[bt * 128:(bt + 1) * 128, :])
                xts_in.append(xt)

            w_raw = persist.tile([128, RKCOLS], bf16)
            nc.sync.dma_start(w_raw[:], wraw_d)
            w_sq = persist.tile([128, RKCOLS], bf16)
            nc.sync.dma_start(w_sq[:], wsq_d)

            ones = persist.tile([1, 128], bf16)
            nc.vector.memset(ones[:], 1.0)

            # full HBM scratch: xT row f = feature f, 512 batch cols bf16
            xt_dram = drampool.tile([F, BL], bf16)
            xt_wview = xt_dram[:].rearrange("(c p) b -> p c b", p=128)

            # --- const broadcast: k=1 matmuls replicate cstb across the
            # 128 partitions; copy to SBUF f32 for the fused evacuation ---
            cst_sb = persist.tile([128, RKCOLS], f32)
            for cc in range(4):
                cstp = cstpool.tile([128, 512], f32)
                nc.tensor.matmul(
                    cstp[:], ones[:],
                    cstb[:, cc * 512:(cc + 1) * 512],
                    start=True, stop=True,
                )
                nc.vector.tensor_copy(
                    cst_sb[:, cc * 512:(cc + 1) * 512], cstp[:]
                )

            # --- phase 1: stream batch tiles into the xT scratch ---
            for bt in range(NT):
                xgb = xgbpool.tile([128, F], bf16)
                nc.scalar.copy(xgb[:], xts_in[bt][:])
                xts = xtspool.tile([128, F], bf16)
                for half in range(2):
                    pt = trpool.tile([128, 512], bf16)
                    for jj in range(4):
                        c = 4 * half + jj
                        nc.tensor.transpose(
                            pt[:, jj * 128:(jj + 1) * 128],
                            xgb[:, c * 128:(c + 1) * 128],
                            ident[:],
                        )
                    nc.vector.tensor_copy(
                        xts[:, half * 512:(half + 1) * 512], pt[:]
                    )
                nc.sync.dma_start(
                    xt_wview[:, :, bt * 128:(bt + 1) * 128],
                    xts[:].rearrange("p (c b) -> p c b", c=8),
                )

            # --- phase 2: 8 per-pair indirect row gathers (full batch) ---
            gts = []
            for p in range(NPAIR):
                gt = gtpool.tile([128, BL], bf16, tag=f"gt{p}")
                nc.gpsimd.indirect_dma_start(
                    out=gt[:],
                    out_offset=None,
                    in_=xt_dram[:],
                    in_offset=bass.IndirectOffsetOnAxis(
                        ap=offs[:, p:p + 1], axis=0,
                    ),
                )
                gts.append(gt)

            # --- phase 3: half-major (2 banks = 4 pairs per half); pure
            # 2-matmul accumulation groups keep the PE stream dense ---
            osbs = []
            for bt in range(NT):
                osb = opool.tile([128, RKCOLS], bf16, tag=f"osb{bt}")
                osbs.append(osb)
            sqs = {}
            for hh in range(2):
                for pp in range(4):
                    p = 4 * hh + pp
                    sq = sqpool.tile([128, BL], bf16, tag=f"sq{p}")
                    if pp % 2 == 0:
                        nc.vector.tensor_mul(sq[:], gts[p][:], gts[p][:])
                    else:
                        nc.scalar.square(sq[:], gts[p][:])
                    sqs[p] = sq
                for bt in range(NT):
                    rs = slice(bt * 128, (bt + 1) * 128)
                    bs = slice(bt * 128, (bt + 1) * 128)
                    for q2 in range(2):
                        q = 2 * hh + q2
                        po = popool.tile([128, 512], f32)
                        for h in range(2):
                            p = 2 * q + h
                            co = slice(h * 256, (h + 1) * 256)
                            wc = slice(p * 256, (p + 1) * 256)
                            nc.tensor.matmul(
                                po[:, co], gts[p][:, bs], w_raw[:, wc],
                                start=True, stop=False,
                            )
                            nc.tensor.matmul(
                                po[:, co], sqs[p][:, bs], w_sq[:, wc],
                                start=False, stop=True,
                            )
                        cs = slice(q * 512, (q + 1) * 512)
                        nc.vector.scalar_tensor_tensor(
                            out=osbs[bt][:, cs],
                            in0=po[:],
                            scalar=1.0,
                            in1=cst_sb[:, cs],
                            op0=mybir.AluOpType.mult,
                            op1=mybir.AluOpType.add,
                        )
                    nc.sync.dma_start(
                        out_d[rs, hh * 1024:(hh + 1) * 1024],
                        osbs[bt][:, hh * 1024:(hh + 1) * 1024]
                    )

    nc.compile()
    _module_cache["nc"] = nc
    return nc


def _prep_params(regions, means, scales):
    """Host folding of the small [R,K,D] params into matmul weights."""
    regions = np.asarray(regions).astype(np.int64)
    means = np.asarray(means, dtype=np.float64)
    scales = np.asarray(scales, dtype=np.float64)

    inv2 = 1.0 / scales**2                                   # [R,K,D]
    wsq_c = -0.5 * inv2                                      # coeff of x^2
    wraw_c = means * inv2                                    # coeff of x
    const = (
        -0.5 * np.sum(means**2 * inv2, axis=-1)
        - np.sum(np.log(scales), axis=-1)
        - 0.5 * D * LOG_2PI
    )                                                        # [R,K]

    # Block-diagonal weight tiles: pair p covers regions 8p..8p+7.
    # Row 16j+d (region-local j in 0..7), col 32j+k.
    wraw = np.zeros((128, RKCOLS), np.float32)
    wsq = np.zeros((128, RKCOLS), np.float32)
    for p in range(NPAIR):
        for j in range(8):
            r = 8 * p + j
            rows = slice(16 * j, 16 * j + 16)
            cols = slice(256 * p + 32 * j, 256 * p + 32 * j + 32)
            wraw[rows, cols] = wraw_c[r].T.astype(np.float32)   # [D, K]
            wsq[rows, cols] = wsq_c[r].T.astype(np.float32)
    wraw = wraw.astype(ml_dtypes.bfloat16)
    wsq = wsq.astype(ml_dtypes.bfloat16)

    cstb = const.reshape(1, -1).astype(ml_dtypes.bfloat16).copy()

    # per-pair gather offsets: gather p, partition j reads xT row
    # perm[128p + j]; table column p holds those offsets.
    perm = regions.reshape(-1).astype(np.int64)              # [1024]
    offs = perm.reshape(NPAIR, 128).T.astype(np.int32).copy()  # [128, 8]

    ident = np.eye(128, dtype=ml_dtypes.bfloat16)
    return wraw, wsq, cstb, offs, ident


def _run(inputs, trace=False, **kwargs):
    x = np.ascontiguousarray(np.asarray(inputs["x"], dtype=np.float32))
    assert x.shape == (B, F), x.shape
    wraw, wsq, cstb, offs, ident = _prep_params(
        inputs["regions"], inputs["means"], inputs["scales"]
    )

    nc = _build_module()
    in_maps = []
    for c in range(NCORES):
        in_maps.append({
            "x": np.ascontiguousarray(x[c * BL:(c + 1) * BL]),
            "wraw": wraw,
            "wsq": wsq,
            "cstb": cstb,
            "offs": offs,
            "ident": ident,
        })
    res = run_bass_kernel_spmd(
        nc, in_maps, core_ids=list(range(NCORES)), trace=trace, **kwargs
    )
    out = np.concatenate(
        [np.asarray(res.results[c]["out"]).astype(np.float32)
         for c in range(NCORES)], axis=0
    ).reshape(B, R, K)
    return out, res


def kernel(**inputs):
    out, _ = _run(inputs, trace=False)
    return out


# revision 14
# speedup vs baseline: 2.6092x; 1.0080x over previous
"""Trainium2 Bass kernel for nn_GaussianLayer (segment_reduce).

Computes ll[b, r, k] = -0.5 * sum_d((x[b, regions[r,d]] - means[r,k,d]) / scales[r,k,d])^2
                       - sum_d log(scales[r,k,d]) - 0.5 * D * log(2*pi)

Strategy (data-parallel over batch across 8 cores, 512 rows each):
  Host folds the small [R,K,D] params into matmul weights:
      ll = Xsq @ Wsq + Xraw @ Wraw + const
  where Xraw[b, (r,d)] = x[b, regions[r,d]] (the gather), Xsq = Xraw^2,
  Wsq = -0.5/scales^2, Wraw = means/scales^2 (block-diagonal per region),
  const[r,k] = -0.5*sum_d(means^2/scales^2) - sum_d log(scales) - 0.5*D*LOG2PI.

  Device: phase 1 streams the 4 batch tiles (DMA x -> ACT cast bf16 ->
  8 PE transposes -> HWDGE DMA straight from PSUM into a full HBM xT
  scratch [1024 feat, 512 batch] bf16).  The feature gather is 8
  indirect DMAs (one per region-pair, offset table [128,1] = one xT row
  per partition) -- no gpsimd Q7 library, whose ~30us load latency
  would dominate.  const is broadcast across partitions once via k=1 PE
  matmuls into PSUM and copied to SBUF; phase 3 runs 2-matmul
  accumulation groups (raw+sq) into 2-bank PSUM tiles which DVE
  evacuates with a fused (po + const -> bf16) scalar_tensor_tensor.
  Output bf16, host upcast.
"""

import os
import sys

for _p in ("/opt/trn_rl_repo", "/root/.axon_site/_ro/trn_rl_repo"):
    if os.path.isdir(_p) and _p not in sys.path:
        sys.path.insert(0, _p)

import numpy as np
import ml_dtypes

import concourse.bass as bass
import concourse.tile as tile
from concourse import bacc, library_config, mybir
from concourse.bass_utils import run_bass_kernel_spmd

LOG_2PI = 1.8378770664093453
B, F = 4096, 1024
R, K, D = 64, 32, 16
NCORES = 8
BL = B // NCORES      # 512 batch rows per core
NT = BL // 128        # 4 batch tiles per core
RKCOLS = R * K        # 2048 output columns
NPAIR = 8             # pair = 8 regions = 128 gathered rows / 256 out cols

_module_cache = {}


def _build_module():
    if "nc" in _module_cache:
        return _module_cache["nc"]

    nc = bacc.Bacc(
        trn_type="TRN2",
        target_bir_lowering=False,
        debug=False,
        enable_asserts=False,
    )
    bf16 = mybir.dt.bfloat16
    f32 = mybir.dt.float32
    i32 = mybir.dt.int32

    x_d = nc.dram_tensor("x", [BL, F], f32, kind="ExternalInput").ap()
    wraw_d = nc.dram_tensor("wraw", [128, RKCOLS], bf16, kind="ExternalInput").ap()
    wsq_d = nc.dram_tensor("wsq", [128, RKCOLS], bf16, kind="ExternalInput").ap()
    cstb_d = nc.dram_tensor("cstb", [1, RKCOLS], bf16, kind="ExternalInput").ap()
    offs_d = nc.dram_tensor("offs", [128, NPAIR], i32, kind="ExternalInput").ap()
    id_d = nc.dram_tensor("ident", [128, 128], bf16, kind="ExternalInput").ap()
    out_d = nc.dram_tensor("out", [BL, RKCOLS], bf16, kind="ExternalOutput").ap()

    with tile.TileContext(nc) as tc:
        with (
            tc.tile_pool(name="persist", bufs=1) as persist,
            tc.tile_pool(name="dram", bufs=1, space="DRAM") as drampool,
            tc.tile_pool(name="xin", bufs=4) as xpool,
            tc.tile_pool(name="xgb", bufs=2) as xgbpool,
            tc.tile_pool(name="trp", bufs=2, space="PSUM") as trpool,
            tc.tile_pool(name="xts", bufs=2) as xtspool,
            tc.tile_pool(name="gt", bufs=1) as gtpool,
            tc.tile_pool(name="sq", bufs=1) as sqpool,
            tc.tile_pool(name="cstp", bufs=1, space="PSUM") as cstpool,
            tc.tile_pool(name="po", bufs=4, space="PSUM") as popool,
            tc.tile_pool(name="osb", bufs=1) as opool,
        ):
            # --- input DMAs: x tiles first (they gate the xT barrier),
            # small persists next, the 1MB weights last (needed ~15us) ---
            xts_in = []
            xt0 = xpool.tile([128, F], f32, tag="x0")
            nc.sync.dma_start(xt0[:], x_d[0:128, :])
            xts_in.append(xt0)

            ident = persist.tile([128, 128], bf16)
            nc.sync.dma_start(ident[:], id_d)
            offs = persist.tile([128, NPAIR], i32)
            nc.sync.dma_start(offs[:], offs_d)
            cstb = persist.tile([1, RKCOLS], bf16)
            nc.sync.dma_start(cstb[:], cstb_d)

            for bt in (1, 2, 3):
                xt = xpool.tile([128, F], f32, tag=f"x{bt}")
                nc.sync.dma_start(xt[:], x_d[bt * 128:(bt + 1) * 128, :])
                xts_in.append(xt)

            w_raw = persist.tile([128, RKCOLS], bf16)
            nc.sync.dma_start(w_raw[:], wraw_d)
            w_sq = persist.tile([128, RKCOLS], bf16)
            nc.sync.dma_start(w_sq[:], wsq_d)

            ones = persist.tile([1, 128], bf16)
            nc.vector.memset(ones[:], 1.0)

            # full HBM scratch: xT row f = feature f, 512 batch cols bf16
            xt_dram = drampool.tile([F, BL], bf16)
            xt_wview = xt_dram[:].rearrange("(c p) b -> p c b", p=128)

            # --- phase 1: stream batch tiles into the xT scratch ---
            for bt in range(NT):
                xgb = xgbpool.tile([128, F], bf16)
                if bt % 2 == 0:
                    nc.scalar.copy(xgb[:], xts_in[bt][:])
                else:
                    nc.gpsimd.tensor_copy(xgb[:], xts_in[bt][:])
                xts = xtspool.tile([128, F], bf16)
                for half in range(2):
                    pt = trpool.tile([128, 512], bf16)
                    for jj in range(4):
                        c = 4 * half + jj
                        nc.tensor.transpose(
                            pt[:, jj * 128:(jj + 1) * 128],
                            xgb[:, c * 128:(c + 1) * 128],
                            ident[:],
                        )
                    nc.vector.tensor_copy(
                        xts[:, half * 512:(half + 1) * 512], pt[:]
                    )
                nc.sync.dma_start(
                    xt_wview[:, :, bt * 128:(bt + 1) * 128],
                    xts[:].rearrange("p (c b) -> p c b", c=8),
                )

            # --- const broadcast: k=1 matmuls replicate cstb across the
            # 128 partitions; copy to SBUF f32 for the fused evacuation ---
            cst_sb = persist.tile([128, RKCOLS], f32)
            for cc in range(4):
                cstp = cstpool.tile([128, 512], f32)
                nc.tensor.matmul(
                    cstp[:], ones[:],
                    cstb[:, cc * 512:(cc + 1) * 512],
                    start=True, stop=True,
                )
                nc.vector.tensor_copy(
                    cst_sb[:, cc * 512:(cc + 1) * 512], cstp[:]
                )

            # --- phase 2: 8 per-pair indirect row gathers (full batch) ---
            gts = []
            for p in range(NPAIR):
                gt = gtpool.tile([128, BL], bf16, tag=f"gt{p}")
                nc.gpsimd.indirect_dma_start(
                    out=gt[:],
                    out_offset=None,
                    in_=xt_dram[:],
                    in_offset=bass.IndirectOffsetOnAxis(
                        ap=offs[:, p:p + 1], axis=0,
                    ),
                )
                gts.append(gt)

            # --- phase 3: half-major (2 banks = 4 pairs per half); pure
            # 2-matmul accumulation groups keep the PE stream dense ---
            osbs = []
            for bt in range(NT):
                osb = opool.tile([128, RKCOLS], bf16, tag=f"osb{bt}")
                osbs.append(osb)
            sqs = {}
            for hh in range(2):
                for pp in range(4):
                    p = 4 * hh + pp
                    sq = sqpool.tile([128, BL], bf16, tag=f"sq{p}")
                    if pp % 2 == 0:
                        nc.vector.tensor_mul(sq[:], gts[p][:], gts[p][:])
                    else:
                        nc.scalar.square(sq[:], gts[p][:])
                    sqs[p] = sq
                for bt in range(NT):
                    rs = slice(bt * 128, (bt + 1) * 128)
                    bs = slice(bt * 128, (bt + 1) * 128)
                    for q2 in range(2):
                        q = 2 * hh + q2
                        po = popool.tile([128, 512], f32)
                        for h in range(2):
                            p = 2 * q + h
                            co = slice(h * 256, (h + 1) * 256)
                            wc = slice(p * 256, (p + 1) * 256)
                            nc.tensor.matmul(
                                po[:, co], gts[p][:, bs], w_raw[:, wc],
                                start=True, stop=False,
                            )
                            nc.tensor.matmul(
                                po[:, co], sqs[p][:, bs], w_sq[:, wc],
                                start=False, stop=True,
                            )
                        cs = slice(q * 512, (q + 1) * 512)
                        nc.vector.scalar_tensor_tensor(
                            out=osbs[bt][:, cs],
                            in0=po[:],
                            scalar=1.0,
                            in1=cst_sb[:, cs],
                            op0=mybir.AluOpType.mult,
                            op1=mybir.AluOpType.add,
                        )
                    eng = nc.sync if hh == 0 else nc.scalar
                    eng.dma_start(
                        out_d[rs, hh * 1024:(hh + 1) * 1024],
                        osbs[bt][:, hh * 1024:(hh + 1) * 1024]
                    )

    nc.compile()
    _module_cache["nc"] = nc
    return nc


def _prep_params(regions, means, scales):
    """Host folding of the small [R,K,D] params into matmul weights."""
    regions = np.asarray(regions).astype(np.int64)
    means = np.asarray(means, dtype=np.float64)
    scales = np.asarray(scales, dtype=np.float64)

    inv2 = 1.0 / scales**2                                   # [R,K,D]
    wsq_c = -0.5 * inv2                                      # coeff of x^2
    wraw_c = means * inv2                                    # coeff of x
    const = (
        -0.5 * np.sum(means**2 * inv2, axis=-1)
        - np.sum(np.log(scales), axis=-1)
        - 0.5 * D * LOG_2PI
    )                                                        # [R,K]

    # Block-diagonal weight tiles: pair p covers regions 8p..8p+7.
    # Row 16j+d (region-local j in 0..7), col 32j+k.
    wraw = np.zeros((128, RKCOLS), np.float32)
    wsq = np.zeros((128, RKCOLS), np.float32)
    for p in range(NPAIR):
        for j in range(8):
            r = 8 * p + j
            rows = slice(16 * j, 16 * j + 16)
            cols = slice(256 * p + 32 * j, 256 * p + 32 * j + 32)
            wraw[rows, cols] = wraw_c[r].T.astype(np.float32)   # [D, K]
            wsq[rows, cols] = wsq_c[r].T.astype(np.float32)
    wraw = wraw.astype(ml_dtypes.bfloat16)
    wsq = wsq.astype(ml_dtypes.bfloat16)

    cstb = const.reshape(1, -1).astype(ml_dtypes.bfloat16).copy()

    # per-pair gather offsets: gather p, partition j reads xT row
    # perm[128p + j]; table column p holds those offsets.
    perm = regions.reshape(-1).astype(np.int64)              # [1024]
    offs = perm.reshape(NPAIR, 128).T.astype(np.int32).copy()  # [128, 8]

    ident = np.eye(128, dtype=ml_dtypes.bfloat16)
    return wraw, wsq, cstb, offs, ident


def _run(inputs, trace=False, **kwargs):
    x = np.ascontiguousarray(np.asarray(inputs["x"], dtype=np.float32))
    assert x.shape == (B, F), x.shape
    wraw, wsq, cstb, offs, ident = _prep_params(
        inputs["regions"], inputs["means"], inputs["scales"]
    )

    nc = _build_module()
    in_maps = []
    for c in range(NCORES):
        in_maps.append({
            "x": np.ascontiguousarray(x[c * BL:(c + 1) * BL]),
            "wraw": wraw,
            "wsq": wsq,
            "cstb": cstb,
            "offs": offs,
            "ident": ident,
        })
    res = run_bass_kernel_spmd(
        nc, in_maps, core_ids=list(range(NCORES)), trace=trace, **kwargs
    )
    out = np.concatenate(
        [np.asarray(res.results[c]["out"]).astype(np.float32)
         for c in range(NCORES)], axis=0
    ).reshape(B, R, K)
    return out, res


def kernel(**inputs):
    out, _ = _run(inputs, trace=False)
    return out
